# revision 22
# baseline (speedup 1.0000x reference)
"""Multi-head attention (S=2048, B=2, D=1024, H=16) on 8 Trainium2 cores.

Sharding: tensor-parallel over heads (4 groups of 4 heads) x data-parallel
over batch (2). Core r handles batch r//4, heads [4*(r%4), 4*(r%4)+4).
Each core projects its 256 channels, runs attention for its 4 heads, applies
its slice of the output projection, and a ReduceScatter over each 4-core
batch group sums the partial outputs and leaves each core with a 512-row
slice of the final [2048, 1024] output.

All matmuls run in fp32r (fp32 rounded to 12 mantissa bits, full PE rate).
Softmax denominators come free from an extra ones-column appended to V in
the PV matmul. V's bias and the output bias are folded out algebraically
and added on the host.
"""
import sys

sys.path.insert(0, "/opt/trn_rl_repo")

import numpy as np
import concourse.bacc as bacc
import concourse.mybir as mybir
from concourse import tile
from concourse.bass_utils import run_bass_kernel_spmd

dt = mybir.dt
AF = mybir.ActivationFunctionType

S, B, D = 2048, 2, 1024
H, DK = 16, 64
NCORES = 8
HC = 4                 # heads per core
CH = HC * DK           # 256 local channels per core
SCALE = np.float32(1.0 / np.sqrt(DK))
GROUPS = [[0, 1, 2, 3], [4, 5, 6, 7]]

TQ = 512               # tq block (matmul free dim)
NTQ = S // TQ          # 4
NKD = D // 128         # 8 contraction tiles for projections
NTK = S // 128         # 16 key tiles

CHUNKS = [1024, 512, 512]              # ReduceScatter chunk row counts
CHUNK_OFF = [0, 1024, 1536]
# chunk index for each 128-row output subtile (16 subtiles)
SUB2CHUNK = [0] * 8 + [1] * 4 + [2] * 4


def round_fp32r(x: np.ndarray) -> np.ndarray:
    """Round fp32 to fp32r (12-bit mantissa, round-to-nearest-even)."""
    u = np.ascontiguousarray(x, dtype=np.float32).view(np.uint32)
    lsb = (u >> 12) & np.uint32(1)
    r = (u + np.uint32(0x7FF) + lsb) & np.uint32(0xFFFFF000)
    return r.view(np.float32)


def build_nc():
    f32, f32r = dt.float32, dt.float32r
    nc = bacc.Bacc("TRN2", target_bir_lowering=False, debug=False,
                   num_devices=NCORES)

    xq = nc.dram_tensor("xq_t", [D, S], f32r, kind="ExternalInput").ap()
    xk = nc.dram_tensor("xk_t", [D, S], f32r, kind="ExternalInput").ap()
    xv = nc.dram_tensor("xv_t", [D, S], f32r, kind="ExternalInput").ap()
    wq = nc.dram_tensor("wq_t", [D, CH], f32r, kind="ExternalInput").ap()
    wk = nc.dram_tensor("wk_t", [D, CH], f32r, kind="ExternalInput").ap()
    wv = nc.dram_tensor("wv_t", [D, CH], f32r, kind="ExternalInput").ap()
    wo = nc.dram_tensor("wo_t", [CH, D], f32r, kind="ExternalInput").ap()
    bq = nc.dram_tensor("bq", [2, 128], f32, kind="ExternalInput").ap()
    bk = nc.dram_tensor("bk", [2, 128], f32, kind="ExternalInput").ap()
    ones = nc.dram_tensor("ones", [128, HC], f32r, kind="ExternalInput").ap()
    # Uneven ReduceScatter chunks (rows of the partial output). The last
    # chunk is small so the only non-overlapped collective is cheap.
    # Chunk c covers global token rows [CHUNK_OFF[c], CHUNK_OFF[c]+CHUNKS[c]);
    # group-rank j receives rows CHUNK_OFF[c] + j*CHUNKS[c]//4 onward.
    out_ext = nc.dram_tensor("out_rs", [S // 4, D], f32,
                             kind="ExternalOutput").ap()

    with tile.TileContext(nc) as tc:
        with tc.tile_pool(name="const", bufs=1) as cp, \
             tc.tile_pool(name="stream", bufs=1) as sp, \
             tc.tile_pool(name="psum", bufs=1, space="PSUM") as pp, \
             tc.tile_pool(name="dram", bufs=1, space="DRAM") as dp:

            # ---- resident weights / biases (DMA in need-order: wq first) ----
            wq_sb = [cp.tile([128, CH], f32r, tag=f"wq{k}", name=f"wq{k}")
                     for k in range(NKD)]
            wk_sb = [cp.tile([128, CH], f32r, tag=f"wk{k}", name=f"wk{k}")
                     for k in range(NKD)]
            wv_sb = [cp.tile([128, CH], f32r, tag=f"wv{k}", name=f"wv{k}")
                     for k in range(NKD)]
            wo_sb = [cp.tile([128, D], f32r, tag=f"wo{k}", name=f"wo{k}")
                     for k in range(2)]
            bq_sb = [cp.tile([128, 1], f32, tag=f"bq{j}", name=f"bq{j}")
                     for j in range(2)]
            bk_sb = [cp.tile([128, 1], f32, tag=f"bk{j}", name=f"bk{j}")
                     for j in range(2)]
            for k in range(NKD):
                nc.scalar.dma_start(wq_sb[k][:], wq[k * 128:(k + 1) * 128, :])
            for j in range(2):
                nc.scalar.dma_start(bq_sb[j][:], bq[j].unsqueeze(1))
                nc.scalar.dma_start(bk_sb[j][:], bk[j].unsqueeze(1))
            for k in range(NKD):
                nc.scalar.dma_start(wk_sb[k][:], wk[k * 128:(k + 1) * 128, :])
            for k in range(NKD):
                nc.scalar.dma_start(wv_sb[k][:], wv[k * 128:(k + 1) * 128, :])
            for k in range(2):
                nc.scalar.dma_start(wo_sb[k][:], wo[k * 128:(k + 1) * 128, :])

            # ---- persistent activations ----
            qc = [cp.tile([128, S], f32r, tag=f"qc{j}", name=f"qc{j}")
                  for j in range(2)]
            kc = [cp.tile([128, S], f32r, tag=f"kc{j}", name=f"kc{j}")
                  for j in range(2)]
            # V tiles: [token128, 4*(64 V + 1 ones)] per key tile
            vt = [cp.tile([128, HC * (DK + 1)], f32r, tag=f"vt{t}",
                          name=f"vt{t}") for t in range(NTK)]
            ctx = [cp.tile([128, S], f32r, tag=f"ctx{j}", name=f"ctx{j}")
                   for j in range(2)]
            ones_sb = cp.tile([128, HC], f32r, tag="ones", name="ones_sb")
            nc.scalar.dma_start(ones_sb[:], ones[:])
            for t in range(NTK):
                vt_view = vt[t][:].rearrange("p (h c) -> p h c", h=HC)
                nc.vector.tensor_copy(vt_view[:, :, DK:DK + 1],
                                      ones_sb[:].unsqueeze(2))

            # ---- Q/K projections: psum[j-tile] [128ch, 512t] = sum_k
            #      wq[k][:, j]   (stationary) . xq[k, t512] (moving) ----
            for x_dram, w_sb, b_sb, dst in ((xq, wq_sb, bq_sb, qc),
                                            (xk, wk_sb, bk_sb, kc)):
                for th in range(2):            # halves of the tq range
                    xts = []
                    for k in range(NKD):
                        for t in range(2):
                            xt = sp.tile([128, TQ], f32r, tag="xs", bufs=20,
                                         name=f"xs{k}_{t}")
                            tq0 = (th * 2 + t) * TQ
                            nc.sync.dma_start(
                                xt[:], x_dram[k * 128:(k + 1) * 128,
                                              tq0:tq0 + TQ])
                            xts.append(xt)
                    for j in range(2):
                        ps = [pp.tile([128, TQ], f32, tag="mm", bufs=2,
                                      name=f"pp{j}_{t}") for t in range(2)]
                        for k in range(NKD):
                            for t in range(2):
                                nc.tensor.matmul(
                                    ps[t][:],
                                    w_sb[k][:, j * 128:(j + 1) * 128],
                                    xts[2 * k + t][:],
                                    start=(k == 0), stop=(k == NKD - 1))
                        for t in range(2):
                            tq0 = (th * 2 + t) * TQ
                            nc.scalar.activation(
                                dst[j][:, tq0:tq0 + TQ], ps[t][:],
                                AF.Identity, bias=b_sb[j][:, 0:1])

            # ---- V projection (choice A): psum [128t, 256ch] = sum_k
            #      xv[k, t128] (stationary) . wv[k] (moving) ----
            for tt in range(4):                # big spans of 4 t-tiles
                xvts = []
                for k in range(NKD):
                    xvt_ = sp.tile([128, TQ], f32r, tag="xs", bufs=20,
                                   name=f"xvt{k}")
                    nc.sync.dma_start(
                        xvt_[:], xv[k * 128:(k + 1) * 128,
                                    tt * TQ:(tt + 1) * TQ])
                    xvts.append(xvt_)
                for ts in range(4):
                    t = tt * 4 + ts
                    pv = pp.tile([128, CH], f32, tag="mm", bufs=2,
                                 name=f"pv{t}")
                    for k in range(NKD):
                        nc.tensor.matmul(
                            pv[:], xvts[k][:, ts * 128:(ts + 1) * 128],
                            wv_sb[k][:],
                            start=(k == 0), stop=(k == NKD - 1))
                    # strided copy into [128, 4, 65][:, :, 0:64]
                    dst_view = vt[t][:].rearrange("p (h c) -> p h c", h=HC)
                    src_view = pv[:].rearrange("p (h c) -> p h c", h=HC)
                    nc.vector.tensor_copy(dst_view[:, :, 0:DK], src_view)

            # ---- attention + output projection ----
            cc_ins = [dp.tile([CHUNKS[c], D], f32, tag=f"ccin{c}",
                              name=f"cc_in{c}") for c in range(len(CHUNKS))]
            cc_outs = [dp.tile([CHUNKS[c] // 4, D], f32, tag=f"ccout{c}",
                               name=f"cc_out{c}") for c in range(len(CHUNKS))]
            for tqi in range(NTQ):
                tq0 = tqi * TQ
                for p in range(2):             # head pairs (2p, 2p+1)
                    cx = [pp.tile([65, TQ], f32, tag="cx", bufs=2,
                                  name=f"cx{p}_{h}") for h in range(2)]
                    for tk in range(NTK):
                        # both heads' scores side by side in one 2-bank tile
                        s1 = pp.tile([128, 2 * TQ], f32, tag="s1", bufs=2,
                                     name=f"s1{tk}")
                        et = sp.tile([128, 2 * TQ], f32r, tag="et", bufs=4,
                                     name=f"et{tk}")
                        for h in range(2):      # adjacent -> row-pack overlap
                            r0 = h * 64
                            nc.tensor.matmul(
                                s1[:, h * TQ:(h + 1) * TQ],
                                kc[p][r0:r0 + 64, tk * 128:(tk + 1) * 128],
                                qc[p][r0:r0 + 64, tq0:tq0 + TQ],
                                start=True, stop=True)
                        # one ACT instruction covers both heads
                        nc.scalar.activation(et[:], s1[:], AF.Exp)
                        for h in range(2):
                            hl = p * 2 + h
                            nc.tensor.matmul(
                                cx[h][:],
                                vt[tk][:, hl * 65:(hl + 1) * 65],
                                et[:, h * TQ:(h + 1) * TQ],
                                start=(tk == 0), stop=(tk == NTK - 1))
                    cxs = []
                    for h in range(2):
                        # evacuate both psums first so the cx slots free up
                        c_ = sp.tile([65, TQ], f32, tag="cxs", bufs=4,
                                     name=f"cxs{p}_{h}")
                        nc.vector.tensor_copy(c_[:], cx[h][:])
                        cxs.append(c_)
                    for h in range(2):
                        rc = sp.tile([1, TQ], f32, tag="rc", bufs=4,
                                     name=f"rc{p}_{h}")
                        nc.vector.reciprocal(rc[:], cxs[h][64:65, :])
                        bc = sp.tile([64, TQ], f32, tag="bc", bufs=4,
                                     name=f"bc{p}_{h}")
                        nc.gpsimd.partition_broadcast(bc[:], rc[:])
                        nc.vector.tensor_mul(
                            ctx[p][h * 64:(h + 1) * 64, tq0:tq0 + TQ],
                            cxs[h][0:64, :], bc[:])
                # output projection for this tq block
                for s4 in range(4):
                    t0 = tq0 + s4 * 128
                    po = [pp.tile([128, TQ], f32, tag="mm", bufs=2,
                                  name=f"po{s4}_{e}") for e in range(2)]
                    for e in range(2):
                        for dv in range(2):
                            nc.tensor.matmul(
                                po[e][:],
                                ctx[dv][:, t0:t0 + 128],
                                wo_sb[dv][:, e * TQ:(e + 1) * TQ],
                                start=(dv == 0), stop=(dv == 1))
                    osb = sp.tile([128, D], f32, tag="ot", bufs=3,
                                  name=f"ot{s4}")
                    for e in range(2):
                        nc.vector.tensor_copy(osb[:, e * TQ:(e + 1) * TQ],
                                              po[e][:])
                    sub = tqi * 4 + s4
                    c = SUB2CHUNK[sub]
                    r0 = sub * 128 - CHUNK_OFF[c]
                    nc.gpsimd.dma_start(cc_ins[c][r0:r0 + 128, :], osb[:])
                    if sub * 128 + 128 == CHUNK_OFF[c] + CHUNKS[c]:
                        # chunk complete: ReduceScatter it (overlaps the
                        # attention compute of the following blocks)
                        nc.gpsimd.collective_compute(
                            "ReduceScatter", mybir.AluOpType.add,
                            replica_groups=GROUPS,
                            ins=[cc_ins[c][:]], outs=[cc_outs[c][:]])
                        o0 = CHUNK_OFF[c] // 4
                        nc.gpsimd.dma_start(
                            out_ext[o0:o0 + CHUNKS[c] // 4, :], cc_outs[c][:])

    nc.finalize()
    return nc


_NC = None


def _get_nc():
    global _NC
    if _NC is None:
        _NC = build_nc()
    return _NC


def make_in_maps(q, k, v, Wq, bq, Wk, bk, Wv, bv, Wo, bo):
    """Shard + precondition full inputs into per-core input maps."""
    xq_b = [round_fp32r(q[:, b, :].T) for b in range(B)]
    xk_b = [round_fp32r(k[:, b, :].T) for b in range(B)]
    xv_b = [round_fp32r(v[:, b, :].T) for b in range(B)]
    in_maps = []
    for r in range(NCORES):
        b = r // 4
        g = r % 4
        ch = slice(g * CH, (g + 1) * CH)
        in_maps.append({
            "xq_t": xq_b[b], "xk_t": xk_b[b], "xv_t": xv_b[b],
            "wq_t": round_fp32r((Wq[ch, :] * SCALE).T),
            "wk_t": round_fp32r(Wk[ch, :].T),
            "wv_t": round_fp32r(Wv[ch, :].T),
            "wo_t": round_fp32r(Wo[:, ch].T),
            "bq": (bq[ch] * SCALE).reshape(2, 128).astype(np.float32),
            "bk": bk[ch].reshape(2, 128).astype(np.float32),
            "ones": np.ones((128, HC), dtype=np.float32),
        })
    return in_maps


def assemble(results, Wo, bv, bo):
    """Gather per-core ReduceScatter slices into the full [S, B, D] output."""
    out = np.empty((S, B, D), dtype=np.float32)
    for r in range(NCORES):
        b = r // 4
        j = r % 4
        for c, (n, off) in enumerate(zip(CHUNKS, CHUNK_OFF)):
            rows = n // 4
            g0 = off + j * rows                  # global token rows
            o0 = off // 4                        # rows within out_rs
            out[g0:g0 + rows, b, :] = results[r]["out_rs"][o0:o0 + rows]
    out += (bo + Wo @ bv).astype(np.float32)
    return out


def run_sharded(inputs, trace=False):
    nc = _get_nc()
    in_maps = make_in_maps(**inputs)
    res = run_bass_kernel_spmd(nc, in_maps, list(range(NCORES)), trace=trace)
    full = assemble(res.results, np.asarray(inputs["Wo"], dtype=np.float32),
                    np.asarray(inputs["bv"], dtype=np.float32),
                    np.asarray(inputs["bo"], dtype=np.float32))
    return full, res


def kernel(**inputs) -> np.ndarray:
    inputs = {k_: np.asarray(v_, dtype=np.float32)
              for k_, v_ in inputs.items()}
    full, _ = run_sharded(inputs)
    return full
